# revision 5
# baseline (speedup 1.0000x reference)
"""Bass/Trainium2 kernel for nn_DegeneratePool: out = x / (H*W + 1e-9).

Pure elementwise scale of a (32, 64, 224, 224) f32 tensor. Data-parallel
across 8 NeuronCores: 4 batches (~51.4 MB) per core, streamed through SBUF
in large contiguous tiles (memory-bound; HWDGE DMAs, DVE multiply).
"""

import numpy as np

import concourse.bacc as bacc
import concourse.mybir as mybir
from concourse.bass_utils import run_bass_kernel_spmd
from concourse.tile import TileContext

N_CORES = 8
B, C, H, W = 32, 64, 224, 224
SCALE = 1.0 / (H * W + 1e-9)

PER_CORE_ELEMS = (B // N_CORES) * C * H * W  # 12,845,056
P = 128
TILE_F = 6272
NTILES = PER_CORE_ELEMS // (P * TILE_F)  # 16
assert NTILES * P * TILE_F == PER_CORE_ELEMS


def _build_nc() -> bacc.Bacc:
    nc = bacc.Bacc("TRN2", target_bir_lowering=False, num_devices=N_CORES)
    x = nc.dram_tensor("x", [NTILES, P, TILE_F], mybir.dt.float32, kind="ExternalInput")
    y = nc.dram_tensor("y", [NTILES, P, TILE_F], mybir.dt.float32, kind="ExternalOutput")

    with TileContext(nc) as tc:
        with tc.tile_pool(name="sbuf", bufs=4) as pool:
            for i in range(NTILES):
                t = pool.tile([P, TILE_F], mybir.dt.float32)
                nc.sync.dma_start(out=t[:], in_=x[i])
                nc.vector.tensor_scalar_mul(t[:], t[:], SCALE)
                nc.sync.dma_start(out=y[i], in_=t[:])
    nc.compile()
    return nc


def kernel(x: np.ndarray) -> np.ndarray:
    assert x.shape == (B, C, H, W) and x.dtype == np.float32
    nc = _build_nc()
    per_core = B // N_CORES
    shards = np.ascontiguousarray(x).reshape(N_CORES, NTILES, P, TILE_F)
    in_maps = [{"x": shards[i]} for i in range(N_CORES)]
    res = run_bass_kernel_spmd(nc, in_maps, core_ids=list(range(N_CORES)))
    out = np.concatenate(
        [r["y"].reshape(per_core, C, H, W) for r in res.results], axis=0
    )
    return out


# revision 6
# speedup vs baseline: 1.6238x; 1.6238x over previous
"""Bass/Trainium2 kernel for nn_DegeneratePool: out = x / (H*W + 1e-9).

Pure elementwise scale of a (32, 64, 224, 224) f32 tensor. Data-parallel
across 8 NeuronCores: 4 batches (~51.4 MB) per core, streamed through SBUF
in large contiguous tiles (memory-bound; HWDGE DMAs, DVE multiply).
"""

import numpy as np

import concourse.bacc as bacc
import concourse.mybir as mybir
from concourse.bass_utils import run_bass_kernel_spmd
from concourse.tile import TileContext

N_CORES = 8
B, C, H, W = 32, 64, 224, 224
SCALE = 1.0 / (H * W + 1e-9)

PER_CORE_ELEMS = (B // N_CORES) * C * H * W  # 12,845,056
P = 128
FREE = PER_CORE_ELEMS // P  # 100,352


def _build_nc(variant: str = "base", tile_f: int = 6272, bufs: int = 4) -> bacc.Bacc:
    ntiles = FREE // tile_f
    assert ntiles * tile_f == FREE, (tile_f, FREE)
    nc = bacc.Bacc("TRN2", target_bir_lowering=False, num_devices=N_CORES)
    x = nc.dram_tensor("x", [ntiles, P, tile_f], mybir.dt.float32, kind="ExternalInput")
    y = nc.dram_tensor("y", [ntiles, P, tile_f], mybir.dt.float32, kind="ExternalOutput")

    with TileContext(nc) as tc:
        if variant == "calib":
            # Near-empty kernel with identical I/O signature, for measuring
            # the fixed per-execution overhead of the run path.
            with tc.tile_pool(name="sbuf", bufs=1) as pool:
                t = pool.tile([P, 512], mybir.dt.float32)
                nc.sync.dma_start(out=t[:], in_=x[0, :, :512])
                nc.vector.tensor_scalar_mul(t[:], t[:], SCALE)
                nc.sync.dma_start(out=y[0, :, :512], in_=t[:])
        elif variant == "base":
            with tc.tile_pool(name="sbuf", bufs=bufs) as pool:
                for i in range(ntiles):
                    t = pool.tile([P, tile_f], mybir.dt.float32)
                    nc.sync.dma_start(out=t[:], in_=x[i])
                    nc.vector.tensor_scalar_mul(t[:], t[:], SCALE)
                    nc.sync.dma_start(out=y[i], in_=t[:])
        elif variant == "scalar_store":
            # loads on SP HWDGE ring, stores on ACT HWDGE ring
            with tc.tile_pool(name="sbuf", bufs=bufs) as pool:
                for i in range(ntiles):
                    t = pool.tile([P, tile_f], mybir.dt.float32)
                    nc.sync.dma_start(out=t[:], in_=x[i])
                    nc.vector.tensor_scalar_mul(t[:], t[:], SCALE)
                    nc.scalar.dma_start(out=y[i], in_=t[:])
        elif variant == "act_mul":
            # multiply on the scalar (ACT) engine instead of DVE
            with tc.tile_pool(name="sbuf", bufs=bufs) as pool:
                for i in range(ntiles):
                    t = pool.tile([P, tile_f], mybir.dt.float32)
                    nc.sync.dma_start(out=t[:], in_=x[i])
                    nc.scalar.mul(t[:], t[:], SCALE)
                    nc.sync.dma_start(out=y[i], in_=t[:])
        elif variant == "split_rings":
            # alternate both loads and stores between the two HWDGE rings
            with tc.tile_pool(name="sbuf", bufs=bufs) as pool:
                for i in range(ntiles):
                    t = pool.tile([P, tile_f], mybir.dt.float32)
                    ld = nc.sync if i % 2 == 0 else nc.scalar
                    st = nc.scalar if i % 2 == 0 else nc.sync
                    ld.dma_start(out=t[:], in_=x[i])
                    nc.vector.tensor_scalar_mul(t[:], t[:], SCALE)
                    st.dma_start(out=y[i], in_=t[:])
        elif variant == "gpsimd":
            # SWDGE path for all DMAs
            with tc.tile_pool(name="sbuf", bufs=bufs) as pool:
                for i in range(ntiles):
                    t = pool.tile([P, tile_f], mybir.dt.float32)
                    nc.gpsimd.dma_start(out=t[:], in_=x[i])
                    nc.vector.tensor_scalar_mul(t[:], t[:], SCALE)
                    nc.gpsimd.dma_start(out=y[i], in_=t[:])
        else:
            raise ValueError(variant)
    nc.compile()
    return nc


def kernel(x: np.ndarray) -> np.ndarray:
    assert x.shape == (B, C, H, W) and x.dtype == np.float32
    nc = _build_nc()
    per_core = B // N_CORES
    ntiles = FREE // 6272
    shards = np.ascontiguousarray(x).reshape(N_CORES, ntiles, P, 6272)
    in_maps = [{"x": shards[i]} for i in range(N_CORES)]
    res = run_bass_kernel_spmd(nc, in_maps, core_ids=list(range(N_CORES)))
    out = np.concatenate(
        [r["y"].reshape(per_core, C, H, W) for r in res.results], axis=0
    )
    return out


# revision 7
# speedup vs baseline: 9.1628x; 5.6427x over previous
"""Bass/Trainium2 kernel for nn_DegeneratePool: out = x / (H*W + 1e-9).

Pure elementwise scale of a (32, 64, 224, 224) f32 tensor. Data-parallel
across 8 NeuronCores: 4 batches (~51.4 MB) per core, streamed through SBUF
in large contiguous tiles (memory-bound; HWDGE DMAs, DVE multiply).
"""

import numpy as np

import concourse.bacc as bacc
import concourse.mybir as mybir
from concourse.bass_utils import run_bass_kernel_spmd
from concourse.tile import TileContext

N_CORES = 8
B, C, H, W = 32, 64, 224, 224
SCALE = 1.0 / (H * W + 1e-9)

PER_CORE_ELEMS = (B // N_CORES) * C * H * W  # 12,845,056
P = 128
FREE = PER_CORE_ELEMS // P  # 100,352


def _build_nc(
    variant: str = "base", tile_f: int = 6272, bufs: int = 4, repeats: int = 1
) -> bacc.Bacc:
    ntiles = FREE // tile_f
    assert ntiles * tile_f == FREE, (tile_f, FREE)
    nc = bacc.Bacc("TRN2", target_bir_lowering=False, num_devices=N_CORES)
    x = nc.dram_tensor("x", [ntiles, P, tile_f], mybir.dt.float32, kind="ExternalInput")
    y = nc.dram_tensor("y", [ntiles, P, tile_f], mybir.dt.float32, kind="ExternalOutput")

    with TileContext(nc) as tc:
        with tc.tile_pool(name="sbuf", bufs=bufs) as pool:
            for _ in range(repeats):
                for i in range(ntiles):
                    if variant == "base":
                        t = pool.tile([P, tile_f], mybir.dt.float32)
                        nc.sync.dma_start(out=t[:], in_=x[i])
                        nc.vector.tensor_scalar_mul(t[:], t[:], SCALE)
                        nc.sync.dma_start(out=y[i], in_=t[:])
                    elif variant == "scalar_store":
                        # loads on SP HWDGE ring, stores on ACT HWDGE ring
                        t = pool.tile([P, tile_f], mybir.dt.float32)
                        nc.sync.dma_start(out=t[:], in_=x[i])
                        nc.vector.tensor_scalar_mul(t[:], t[:], SCALE)
                        nc.scalar.dma_start(out=y[i], in_=t[:])
                    elif variant == "act_mul":
                        # multiply on the scalar (ACT) engine instead of DVE
                        t = pool.tile([P, tile_f], mybir.dt.float32)
                        nc.sync.dma_start(out=t[:], in_=x[i])
                        nc.scalar.mul(t[:], t[:], SCALE)
                        nc.sync.dma_start(out=y[i], in_=t[:])
                    elif variant == "split_rings":
                        t = pool.tile([P, tile_f], mybir.dt.float32)
                        ld = nc.sync if i % 2 == 0 else nc.scalar
                        st = nc.scalar if i % 2 == 0 else nc.sync
                        ld.dma_start(out=t[:], in_=x[i])
                        nc.vector.tensor_scalar_mul(t[:], t[:], SCALE)
                        st.dma_start(out=y[i], in_=t[:])
                    elif variant == "gpsimd":
                        t = pool.tile([P, tile_f], mybir.dt.float32)
                        nc.gpsimd.dma_start(out=t[:], in_=x[i])
                        nc.vector.tensor_scalar_mul(t[:], t[:], SCALE)
                        nc.gpsimd.dma_start(out=y[i], in_=t[:])
                    else:
                        raise ValueError(variant)
    nc.compile()
    return nc


def kernel(x: np.ndarray) -> np.ndarray:
    assert x.shape == (B, C, H, W) and x.dtype == np.float32
    nc = _build_nc()
    per_core = B // N_CORES
    ntiles = FREE // 6272
    shards = np.ascontiguousarray(x).reshape(N_CORES, ntiles, P, 6272)
    in_maps = [{"x": shards[i]} for i in range(N_CORES)]
    res = run_bass_kernel_spmd(nc, in_maps, core_ids=list(range(N_CORES)))
    out = np.concatenate(
        [r["y"].reshape(per_core, C, H, W) for r in res.results], axis=0
    )
    return out
